# revision 1
# baseline (speedup 1.0000x reference)
"""Box-from-mask kernel for Trainium2 (8 NeuronCores, SPMD data-parallel).

Problem: masks [100, 800, 1280] f32 -> boxes [100, 2, 2] f32 where
box[n] = [[xmin, ymin], [xmax, ymax]] of {(y, x) : masks[n, y, x] > 0.5},
with empty-mask sentinels xmin=W, ymin=H, xmax=-1, ymax=-1.

Sharding: each mask is split into two 400-row halves ("units"); the 200
units spread exactly 25 per core (no padding, and the per-core shard is a
copy-free view of masks.reshape(200, 400, 1280)). Host-side combine of the
two halves is an elementwise max thanks to the masked-max formulation.

Per-core device pipeline, per [128, 1280] row-tile of each unit:
  - DVE tensor_scalar(is_gt 0.5) -> 0/1 bf16 tile with accum_out(max) giving
    per-row "any", or (alternating) ACT relu(x-0.5) -> bf16 with accum_out
    (sum) whose positivity is an exact any-indicator. One pass per element.
  - PE one-hot-column matmul over the binary tile accumulates per-column
    counts into that unit's PSUM row across the unit's row-tiles.
The 16-row runt tiles of up to 8 consecutive units are packed into a single
128-partition tile; a block-selector weight matrix routes each 16-row band
to its own PSUM row. Row/col "any" vectors become min/max indices via the
masked-max trick (max of any*(D - idx) and any*(idx + 1)); the final
cross-partition max folds and the affine fixup happen host-side on ~7KB.
"""

import sys

for _p in ("/opt/trn_rl_repo", "/opt/pypackages"):
    if _p not in sys.path:
        sys.path.append(_p)

import ml_dtypes
import numpy as np

import concourse.bass as bass
import concourse.tile as tile
from concourse import bacc, mybir
from concourse.bass_utils import run_bass_kernel_spmd

N, H, W = 100, 800, 1280
N_CORES = 8
THRESHOLD = 0.5

# unit scheme: "half" = 200 half-masks, 25/core, no padding;
#              "full" = whole masks padded 100 -> 104, 13/core
UNIT_SCHEME = "half"
if UNIT_SCHEME == "half":
    HALVES = 2
else:
    HALVES = 1
HU = H // HALVES  # rows per unit
NU = HALVES * N  # real units
KU = -(-NU // N_CORES)  # units per core (padded if uneven)
NU_PAD = KU * N_CORES

fp32 = mybir.dt.float32
fp16 = mybir.dt.float16
bf16 = mybir.dt.bfloat16
Op = mybir.AluOpType


def _chunks(w):
    return [(c, min(512, w - c)) for c in range(0, w, 512)]


PACK_RUNTS = False
TILES_PER_DMA = 3  # full 128-row tiles fetched per wide DMA
RAW_BUFS = 6
BIN_BUFS = 12


def _shape(k, h):
    """Derived tiling/grouping parameters for k units of h rows."""
    ft = h // 128  # full 128-row tiles per unit
    rr = h % 128  # runt rows per unit
    nt_u = ft + (1 if rr and not PACK_RUNTS else 0)  # rowany cols per unit
    ntp = nt_u + (nt_u & 1)  # padded even
    g0 = (k + 1) // 2
    groups = [(0, g0)] + ([(g0, k - g0)] if k > g0 else [])
    cap = (128 // rr) if rr else 0  # units per runt pack
    packs = []  # (group, local_offset, n_units)
    if rr and PACK_RUNTS:
        for g, (gstart, gsz) in enumerate(groups):
            o = 0
            while o < gsz:
                packs.append((g, o, min(cap, gsz - o)))
                o += cap
    gmax = max(gsz for _, gsz in groups)
    return ft, rr, ntp, groups, packs, gmax


def build_program(k=KU, h=HU, w=W):
    """One-core Bass/Tile program; run SPMD on all 8 cores."""
    chunks = _chunks(w)
    ft, rr, ntp, groups, packs, gmax = _shape(k, h)
    npk = len(packs)
    rww = max(2, min(TILES_PER_DMA, max(ft, 1))) * w  # raw slot width

    nc = bacc.Bacc(
        "TRN2", target_bir_lowering=False, debug=False, enable_asserts=False
    )
    masks = nc.dram_tensor("masks", [k, h, w], fp32, kind="ExternalInput").ap()
    c1r = nc.dram_tensor("c1r", [128, k * ntp], fp16, kind="ExternalInput").ap()
    c2r = nc.dram_tensor("c2r", [128, k * ntp], fp16, kind="ExternalInput").ap()
    c1x = nc.dram_tensor("c1x", [gmax, w], fp16, kind="ExternalInput").ap()
    c2x = nc.dram_tensor("c2x", [gmax, w], fp16, kind="ExternalInput").ap()
    # ohs[p, c] = 1 iff c == gmax-1: window ohs[:, gmax-1-jl : gmax-1-jl+gsz]
    # is a one-hot-column matrix selecting PSUM output partition jl.
    ohs = nc.dram_tensor("ohs", [128, 2 * gmax], bf16, kind="ExternalInput").ap()
    outs = {}
    rows_out = nc.dram_tensor(
        "rows_out", [128, 2 * k], fp16, kind="ExternalOutput"
    ).ap()
    cols_out = nc.dram_tensor("cols_out", [k, 2], fp16, kind="ExternalOutput").ap()
    if npk:
        # bsel[p, c] = 1 iff c == gmax + p // rr: window
        # bsel[:u*rr, gmax-o : gmax-o+gsz] routes 16-row band q to PSUM row o+q
        bsel = nc.dram_tensor("bsel", [128, 2 * gmax], bf16, kind="ExternalInput").ap()
        c1p = nc.dram_tensor("c1p", [128, npk], fp16, kind="ExternalInput").ap()
        c2p = nc.dram_tensor("c2p", [128, npk], fp16, kind="ExternalInput").ap()
        rr16_out = nc.dram_tensor(
            "rr16_out", [128, 2 * npk], fp16, kind="ExternalOutput"
        ).ap()

    with tile.TileContext(nc) as tc:
        with (
            tc.tile_pool(name="raw", bufs=RAW_BUFS) as rawp,
            tc.tile_pool(name="bin", bufs=BIN_BUFS) as binp,
            tc.tile_pool(name="consts", bufs=1) as constp,
            tc.tile_pool(name="small", bufs=2) as smallp,
            tc.tile_pool(name="psum", bufs=1, space="PSUM") as psump,
        ):
            # consts ride gpsimd SWDGE queues so SP's HWDGE queues start
            # streaming mask tiles immediately
            c1r_t = constp.tile([128, k * ntp], fp16)
            nc.gpsimd.dma_start(c1r_t[:], c1r)
            c2r_t = constp.tile([128, k * ntp], fp16)
            nc.gpsimd.dma_start(c2r_t[:], c2r)
            c1x_t = constp.tile([gmax, w], fp16)
            nc.gpsimd.dma_start(c1x_t[:], c1x)
            c2x_t = constp.tile([gmax, w], fp16)
            nc.gpsimd.dma_start(c2x_t[:], c2x)
            ohs_t = constp.tile([128, 2 * gmax], bf16)
            nc.gpsimd.dma_start(ohs_t[:], ohs)
            if npk:
                bsel_t = constp.tile([128, 2 * gmax], bf16)
                nc.gpsimd.dma_start(bsel_t[:], bsel)
                c1p_t = constp.tile([128, npk], fp16)
                nc.gpsimd.dma_start(c1p_t[:], c1p)
                c2p_t = constp.tile([128, npk], fp16)
                nc.gpsimd.dma_start(c2p_t[:], c2p)
                rowany16 = constp.tile([128, npk], fp32)
                nc.gpsimd.memset(rowany16[:], 0.0)
                rr16 = constp.tile([128, 2 * npk], fp16)

            rowany = constp.tile([128, k * ntp], fp32)
            nc.gpsimd.memset(rowany[:], 0.0)
            negh = constp.tile([128, 1], fp32)
            nc.gpsimd.memset(negh[:], -THRESHOLD)
            rrt = constp.tile([128, 2 * k], fp16)
            cc = [
                [
                    psump.tile([gsz, cw], fp32, name=f"cc{g}_{ci}", tag=f"cc{g}_{ci}")
                    for ci, (_, cw) in enumerate(chunks)
                ]
                for g, (_, gsz) in enumerate(groups)
            ]

            eng_flip = [0]

            def binarize(out_b, rv, nr, acc):
                """One elementwise pass: binary tile for PE + row-any accum."""
                if eng_flip[0] % 2 == 0:
                    nc.vector.tensor_scalar(
                        out=out_b[:nr, :],
                        in0=rv,
                        scalar1=THRESHOLD,
                        scalar2=None,
                        op0=Op.is_gt,
                        op1=Op.max,
                        accum_out=acc,
                    )
                else:
                    nc.scalar.activation(
                        out=out_b[:nr, :],
                        in_=rv,
                        func=mybir.ActivationFunctionType.Relu,
                        bias=negh[:nr, :],
                        scale=1.0,
                        accum_out=acc,
                    )
                eng_flip[0] += 1

            def emit_cols_tail(g, gstart, gsz):
                """(count > 0) * iota per chunk, then max-reduce along X."""
                csc1 = constp.tile([gsz, w], fp16, name=f"csc1_{g}")
                csc2 = constp.tile([gsz, w], fp16, name=f"csc2_{g}")
                cr = constp.tile([gsz, 2], fp16, name=f"cr_{g}")
                for ci, (c0, cw) in enumerate(chunks):
                    nc.vector.scalar_tensor_tensor(
                        out=csc1[:, c0 : c0 + cw],
                        in0=cc[g][ci][:, :],
                        scalar=0.0,
                        in1=c1x_t[0:gsz, c0 : c0 + cw],
                        op0=Op.is_gt,
                        op1=Op.mult,
                    )
                    nc.vector.scalar_tensor_tensor(
                        out=csc2[:, c0 : c0 + cw],
                        in0=cc[g][ci][:, :],
                        scalar=0.0,
                        in1=c2x_t[0:gsz, c0 : c0 + cw],
                        op0=Op.is_gt,
                        op1=Op.mult,
                    )
                nc.vector.tensor_reduce(
                    out=cr[:, 0:1], in_=csc1[:], axis=mybir.AxisListType.X, op=Op.max
                )
                nc.vector.tensor_reduce(
                    out=cr[:, 1:2], in_=csc2[:], axis=mybir.AxisListType.X, op=Op.max
                )
                # gpsimd SWDGE: the in-order SP queue must not wait on cr
                nc.gpsimd.dma_start(cols_out[gstart : gstart + gsz, :], cr[:])

            def emit_pack(pk, g, gstart, gsz, o, u, is_last):
                """Runt rows of units [gstart+o, gstart+o+u) as one packed tile."""
                j0 = gstart + o
                nr = u * rr
                raw = rawp.tile([128, rww], fp32, tag="raw")
                # shapes differ but element iteration orders align exactly:
                # sbuf partition q*rr + r <- dram (unit j0+q, row ft*128+r)
                nc.sync.dma_start(
                    raw[:nr, :w],
                    masks[j0 : j0 + u, ft * 128 : h, :],
                )
                b = binp.tile([128, w], bf16, tag="b")
                binarize(b, raw[:nr, :w], nr, rowany16[:nr, pk : pk + 1])
                for ci, (c0, cw) in enumerate(chunks):
                    nc.tensor.matmul(
                        cc[g][ci][:, :],
                        bsel_t[:nr, gmax - o : gmax - o + gsz],
                        b[:nr, c0 : c0 + cw],
                        start=False,
                        stop=is_last,
                    )
                nc.vector.scalar_tensor_tensor(
                    out=rr16[:, 2 * pk : 2 * pk + 1],
                    in0=rowany16[:, pk : pk + 1],
                    scalar=0.0,
                    in1=c1p_t[:, pk : pk + 1],
                    op0=Op.is_gt,
                    op1=Op.mult,
                )
                nc.vector.scalar_tensor_tensor(
                    out=rr16[:, 2 * pk + 1 : 2 * pk + 2],
                    in0=rowany16[:, pk : pk + 1],
                    scalar=0.0,
                    in1=c2p_t[:, pk : pk + 1],
                    op0=Op.is_gt,
                    op1=Op.mult,
                )

            for g, (gstart, gsz) in enumerate(groups):
                gpacks = [
                    (pki, o, u) for pki, (pg, o, u) in enumerate(packs) if pg == g
                ]
                # per-unit row tiles: ft full 128-row tiles (DMA'd in pairs)
                # plus, when runts aren't packed, the trailing rr-row tile
                utiles = [(t * 128, 128) for t in range(ft)]
                if rr and not PACK_RUNTS:
                    utiles.append((ft * 128, rr))
                last_t = len(utiles) - 1
                for jl in range(gsz):
                    j = gstart + jl
                    tp = 0
                    while tp < len(utiles):
                        # consecutive full 128-row tiles fetched in one DMA
                        nfull = 0
                        while (
                            nfull < TILES_PER_DMA
                            and tp + nfull < len(utiles)
                            and utiles[tp + nfull][1] == 128
                        ):
                            nfull += 1
                        ngrp = max(nfull, 1)
                        raw = rawp.tile([128, rww], fp32, tag="raw")
                        r0, nr0 = utiles[tp]
                        if nfull >= 2:
                            nc.sync.dma_start(
                                raw[:, : nfull * w],
                                masks[j, r0 : r0 + nfull * 128, :].rearrange(
                                    "(a p) x -> p a x", p=128
                                ),
                            )
                        else:
                            nc.sync.dma_start(
                                raw[:nr0, :w], masks[j, r0 : r0 + nr0, :]
                            )
                        for i in range(ngrp):
                            t = tp + i
                            nr = utiles[t][1]
                            b = binp.tile([128, w], bf16, tag="b")
                            binarize(
                                b,
                                raw[:nr, i * w : i * w + w],
                                nr,
                                rowany[:nr, j * ntp + t : j * ntp + t + 1],
                            )
                            for ci, (c0, cw) in enumerate(chunks):
                                nc.tensor.matmul(
                                    cc[g][ci][:, :],
                                    ohs_t[:nr, gmax - 1 - jl : gmax - 1 - jl + gsz],
                                    b[:nr, c0 : c0 + cw],
                                    start=(jl == 0 and t == 0),
                                    stop=(
                                        (not rr or not PACK_RUNTS)
                                        and jl == gsz - 1
                                        and t == last_t
                                    ),
                                )
                        tp += ngrp
                    # emit any runt pack ending at this unit
                    for pki, o, u in gpacks:
                        if o + u - 1 == jl:
                            emit_pack(
                                pki, g, gstart, gsz, o, u,
                                is_last=(jl == gsz - 1),
                            )
                # batched rows tail for the whole group:
                # rrt[:, j] = max_t((any>0) * (H - idx)), rrt[:, k+j] = ...(idx+1)
                gcols = slice(gstart * ntp, (gstart + gsz) * ntp)
                for fl, cvec in enumerate((c1r_t, c2r_t)):
                    sc = smallp.tile(
                        [128, gsz * ntp], fp16, name=f"sc_{g}_{fl}", tag=f"sc{fl}"
                    )
                    nc.vector.scalar_tensor_tensor(
                        out=sc[:],
                        in0=rowany[:, gcols],
                        scalar=0.0,
                        in1=cvec[:, gcols],
                        op0=Op.is_gt,
                        op1=Op.mult,
                    )
                    nc.vector.tensor_reduce(
                        out=rrt[:, fl * k + gstart : fl * k + gstart + gsz],
                        in_=sc[:].rearrange("p (j t) -> p j t", t=ntp),
                        axis=mybir.AxisListType.X,
                        op=Op.max,
                    )
                emit_cols_tail(g, gstart, gsz)

            # the 128-partition max folds of rrt/rr16 happen host-side (~7KB)
            nc.gpsimd.dma_start(rows_out, rrt[:])
            if npk:
                nc.gpsimd.dma_start(rr16_out, rr16[:])

    nc.compile()
    return nc


def make_consts(k=KU, h=HU, w=W, h_full=H, bases=None):
    """Per-core constant tensors. bases[j] = global row offset of unit j."""
    ft, rr, ntp, groups, packs, gmax = _shape(k, h)
    npk = len(packs)
    if bases is None:
        bases = np.zeros(k, np.int64)
    p = np.arange(128)

    c1r = np.zeros((128, k * ntp), np.float16)
    c2r = np.zeros((128, k * ntp), np.float16)
    for j in range(k):
        for t in range(ft):
            idx = bases[j] + t * 128 + p
            c1r[:, j * ntp + t] = h_full - idx
            c2r[:, j * ntp + t] = idx + 1
        if rr and not PACK_RUNTS:
            idx = bases[j] + ft * 128 + p
            valid = p < rr
            c1r[:, j * ntp + ft] = np.where(valid, h_full - idx, 0)
            c2r[:, j * ntp + ft] = np.where(valid, idx + 1, 0)

    x = np.arange(w)
    c1x = np.broadcast_to((w - x).astype(np.float16), (gmax, w)).copy()
    c2x = np.broadcast_to((x + 1).astype(np.float16), (gmax, w)).copy()
    ohs = np.zeros((128, 2 * gmax), ml_dtypes.bfloat16)
    ohs[:, gmax - 1] = 1
    consts = {"c1r": c1r, "c2r": c2r, "c1x": c1x, "c2x": c2x, "ohs": ohs}

    if npk:
        bsel = np.zeros((128, 2 * gmax), ml_dtypes.bfloat16)
        bsel[p, np.minimum(gmax + p // rr, 2 * gmax - 1)] = 1
        # partitions whose band index exceeds any real pack keep a 1 in some
        # column, but their rhs rows are only read up to u*rr partitions
        c1p = np.zeros((128, npk), np.float16)
        c2p = np.zeros((128, npk), np.float16)
        for pki, (g, o, u) in enumerate(packs):
            gstart = groups[g][0]
            q = p // rr
            valid = q < u
            j = gstart + o + np.minimum(q, u - 1)
            idx = bases[j] + ft * 128 + (p % rr)
            c1p[:, pki] = np.where(valid, h_full - idx, 0)
            c2p[:, pki] = np.where(valid, idx + 1, 0)
        consts.update({"bsel": bsel, "c1p": c1p, "c2p": c2p})
    return consts


def postprocess(results, k=KU, h=HU, w=W, h_full=H, halves_per_box=2):
    """Per-core outputs -> boxes [n_units/halves, 2, 2] f32."""
    ft, rr, ntp, groups, packs, gmax = _shape(k, h)
    npk = len(packs)
    ncores = len(results)
    v1 = np.zeros(ncores * k, np.float64)  # h_full - ymin (0 if empty)
    v2 = np.zeros(ncores * k, np.float64)  # ymax + 1
    u1 = np.zeros(ncores * k, np.float64)  # w - xmin
    u2 = np.zeros(ncores * k, np.float64)  # xmax + 1
    for c, r in enumerate(results):
        rows = np.asarray(r["rows_out"], np.float64).max(axis=0)  # [2k]
        sl = slice(c * k, (c + 1) * k)
        v1[sl] = rows[:k]
        v2[sl] = rows[k:]
        cols = np.asarray(r["cols_out"], np.float64)
        u1[sl] = cols[:, 0]
        u2[sl] = cols[:, 1]
        if npk:
            r16 = np.asarray(r["rr16_out"], np.float64)  # [128, 2*npk]
            for pki, (g, o, u) in enumerate(packs):
                gstart = groups[g][0]
                for q in range(u):
                    j = c * k + gstart + o + q
                    band = slice(q * rr, (q + 1) * rr)
                    v1[j] = max(v1[j], r16[band, 2 * pki].max())
                    v2[j] = max(v2[j], r16[band, 2 * pki + 1].max())
    # combine the halves of each box (elementwise max), then affine fixup
    nb = ncores * k // halves_per_box
    v1 = v1.reshape(nb, halves_per_box).max(axis=1)
    v2 = v2.reshape(nb, halves_per_box).max(axis=1)
    u1 = u1.reshape(nb, halves_per_box).max(axis=1)
    u2 = u2.reshape(nb, halves_per_box).max(axis=1)
    boxes = np.empty((nb, 2, 2), np.float32)
    boxes[:, 0, 0] = w - u1  # xmin
    boxes[:, 0, 1] = h_full - v1  # ymin
    boxes[:, 1, 0] = u2 - 1  # xmax
    boxes[:, 1, 1] = v2 - 1  # ymax
    return boxes


_cache = {}


def _get_program():
    if "nc" not in _cache:
        _cache["nc"] = build_program()
        _cache["consts"] = [
            make_consts(bases=unit_bases(c)) for c in range(N_CORES)
        ]
    return _cache["nc"], _cache["consts"]


def unit_bases(core):
    """Global row offset of each unit on this core."""
    u = core * KU + np.arange(KU)
    return (u % HALVES) * HU


def make_in_maps(masks):
    """Per-core shards of the mask units + consts."""
    masks = np.ascontiguousarray(np.asarray(masks, dtype=np.float32))
    _, consts = _get_program()
    units = masks.reshape(NU, HU, W)
    if NU_PAD > NU:
        units = np.concatenate(
            [units, np.zeros((NU_PAD - NU, HU, W), np.float32)], axis=0
        )
    return [
        {"masks": units[c * KU : (c + 1) * KU], **consts[c]}
        for c in range(N_CORES)
    ]


def kernel(masks):
    nc, _ = _get_program()
    in_maps = make_in_maps(masks)
    res = run_bass_kernel_spmd(nc, in_maps, core_ids=list(range(N_CORES)))
    boxes = postprocess(res.results, halves_per_box=HALVES)
    return boxes[:N]



# revision 2
# speedup vs baseline: 1.0038x; 1.0038x over previous
"""Box-from-mask kernel for Trainium2 (8 NeuronCores, SPMD data-parallel).

Problem: masks [100, 800, 1280] f32 -> boxes [100, 2, 2] f32 where
box[n] = [[xmin, ymin], [xmax, ymax]] of {(y, x) : masks[n, y, x] > 0.5},
with empty-mask sentinels xmin=W, ymin=H, xmax=-1, ymax=-1.

Sharding: the flattened row axis (100*800 = 80000 rows of 1280 px) splits
into 8 contiguous shards of 10000 rows (= 25 half-mask "units" of 400 rows
each). Each core streams its shard once; the measured bottleneck is the
~358 GB/s per-core HBM read cap, so everything else hides under the stream.

Device pipeline per core, per DMA chunk of S sub-rows (partition p holds S
consecutive rows, fully contiguous in HBM -> one 5*S KB run per partition):
  - per sub-row i: one elementwise pass (DVE is_gt / ACT relu alternating)
    produces a 0/1-ish bf16 tile whose accum_out gives per-row "any";
  - PE matmul with a per-sub-row 25-column routing weight accumulates
    per-column counts into each row's unit, one PSUM accumulation group
    spanning the whole stream.
The final 16 rows (10000 = 78*128 + 16) are fetched as a [128, 160] tile
(8 partitions per row) and routed by an (x-phase -> PSUM row) weight into a
separate [8, 160] PSUM tile. The trailing DMA chunks taper (6,...,6,3,2,1
sub-rows) so the post-stream pipeline drain is short. Row/col "any"
bitmaps (~107 KB/core) ship to the host, which computes the min/max box
coordinates exactly in numpy.
"""

import sys

for _p in ("/opt/trn_rl_repo", "/opt/pypackages"):
    if _p not in sys.path:
        sys.path.append(_p)

import ml_dtypes
import numpy as np

import concourse.bass as bass
import concourse.tile as tile
from concourse import bacc, mybir
from concourse.bass_utils import run_bass_kernel_spmd

N, H, W = 100, 800, 1280
N_CORES = 8
THRESHOLD = 0.5

HU = 400  # rows per unit (half mask)
K = 25  # units per core
R = K * HU  # rows per core (10000)
CHUNKS = [6] * 12 + [3, 2, 1]  # sub-rows per stream DMA chunk
NSUB = sum(CHUNKS)  # 78 full 128-partition sub-rows
RUNT = R - 128 * NSUB  # 16 trailing rows, fetched as [128, RUNT*W/128]
RW = RUNT * W // 128  # 160 elems/partition, 8 partitions per row
MAXS = max(CHUNKS)

fp32 = mybir.dt.float32
fp16 = mybir.dt.float16
bf16 = mybir.dt.bfloat16
Op = mybir.AluOpType

_chunk_cols = [(c, min(512, W - c)) for c in range(0, W, 512)]


def build_program():
    """One-core Bass/Tile program; run SPMD on all 8 cores."""
    nc = bacc.Bacc(
        "TRN2", target_bir_lowering=False, debug=False, enable_asserts=False
    )
    masks = nc.dram_tensor("masks", [R, W], fp32, kind="ExternalInput").ap()
    # wmat[:, s*K:(s+1)*K] routes sub-row s's partitions to their unit's
    # PSUM row; wmat[:, NSUB*K:] routes the runt tile's x-phases.
    wmat = nc.dram_tensor(
        "wmat", [128, NSUB * K + 8], bf16, kind="ExternalInput"
    ).ap()
    rowany_out = nc.dram_tensor(
        "rowany_out", [128, NSUB + 2], fp32, kind="ExternalOutput"
    ).ap()
    colany_out = nc.dram_tensor("colany_out", [K, W], fp16, kind="ExternalOutput").ap()
    runt_out = nc.dram_tensor("runt_out", [8, RW], fp16, kind="ExternalOutput").ap()

    with tile.TileContext(nc) as tc:
        with (
            tc.tile_pool(name="raw", bufs=4) as rawp,
            tc.tile_pool(name="bin", bufs=12) as binp,
            tc.tile_pool(name="consts", bufs=1) as constp,
            tc.tile_pool(name="psum", bufs=1, space="PSUM") as psump,
        ):
            # consts ride gpsimd SWDGE queues so SP's HWDGE queues start
            # streaming mask chunks immediately
            wmat_t = constp.tile([128, NSUB * K + 8], bf16)
            nc.gpsimd.dma_start(wmat_t[:], wmat)
            rowany = constp.tile([128, NSUB + 2], fp32)
            nc.gpsimd.memset(rowany[:], 0.0)
            negh = constp.tile([128, 1], fp32)
            nc.gpsimd.memset(negh[:], -THRESHOLD)
            colany_sb = constp.tile([K, W], fp16)
            runt_sb = constp.tile([8, RW], fp16)
            cc = [
                psump.tile([K, cw], fp32, name=f"cc{ci}", tag=f"cc{ci}")
                for ci, (_, cw) in enumerate(_chunk_cols)
            ]
            ccr = psump.tile([8, RW], fp32, name="ccr", tag="ccr")

            eng_flip = [0]

            def binarize(out_b, rv, acc):
                """One elementwise pass: binary tile for PE + row-any accum."""
                if eng_flip[0] % 2 == 0:
                    nc.vector.tensor_scalar(
                        out=out_b,
                        in0=rv,
                        scalar1=THRESHOLD,
                        scalar2=None,
                        op0=Op.is_gt,
                        op1=Op.max,
                        accum_out=acc,
                    )
                else:
                    nc.scalar.activation(
                        out=out_b,
                        in_=rv,
                        func=mybir.ActivationFunctionType.Relu,
                        bias=negh[:, :],
                        scale=1.0,
                        accum_out=acc,
                    )
                eng_flip[0] += 1

            s = 0
            base = 0
            for S in CHUNKS:
                raw = rawp.tile([128, MAXS * W], fp32, tag="raw")
                nc.sync.dma_start(
                    raw[:, : S * W],
                    masks[base : base + 128 * S, :].rearrange(
                        "(p a) x -> p a x", a=S
                    ),
                )
                for i in range(S):
                    b = binp.tile([128, W], bf16, tag="b")
                    binarize(b[:], raw[:, i * W : (i + 1) * W], rowany[:, s : s + 1])
                    for ci, (c0, cw) in enumerate(_chunk_cols):
                        nc.tensor.matmul(
                            cc[ci][:, :],
                            wmat_t[:, s * K : (s + 1) * K],
                            b[:, c0 : c0 + cw],
                            start=(s == 0),
                            stop=(s == NSUB - 1),
                        )
                    s += 1
                base += 128 * S

            # trailing 16 rows as [128, 160]: partition p = row base + p//8,
            # x in [160*(p%8), 160*(p%8)+160)
            raw = rawp.tile([128, MAXS * W], fp32, tag="raw")
            nc.sync.dma_start(
                raw[:, :RW],
                masks[base:R, :].rearrange("y (u a) -> (y u) a", u=128 // RUNT),
            )
            br = binp.tile([128, W], bf16, tag="b")
            binarize(br[:, :RW], raw[:, :RW], rowany[:, NSUB : NSUB + 1])
            nc.tensor.matmul(
                ccr[:, :],
                wmat_t[:, NSUB * K : NSUB * K + 8],
                br[:, :RW],
                start=True,
                stop=True,
            )

            # (count > 0) bitmaps; host does all min/max index math
            for ci, (c0, cw) in enumerate(_chunk_cols):
                nc.vector.tensor_scalar(
                    out=colany_sb[:, c0 : c0 + cw],
                    in0=cc[ci][:, :],
                    scalar1=0.0,
                    scalar2=None,
                    op0=Op.is_gt,
                )
            nc.vector.tensor_scalar(
                out=runt_sb[:],
                in0=ccr[:, :],
                scalar1=0.0,
                scalar2=None,
                op0=Op.is_gt,
            )
            # outputs on the ACT HWDGE ring: fast descriptor gen, and the
            # SP ring (mask stream) plus gpsimd (consts) stay untouched
            nc.scalar.dma_start(rowany_out, rowany[:])
            nc.scalar.dma_start(colany_out, colany_sb[:])
            nc.scalar.dma_start(runt_out, runt_sb[:])

    nc.compile()
    return nc


def make_wmat():
    """Routing weights: one-hot unit id per (sub-row, partition) + runt."""
    wmat = np.zeros((128, NSUB * K + 8), ml_dtypes.bfloat16)
    p = np.arange(128)
    s = 0
    base = 0
    for S in CHUNKS:
        for i in range(S):
            units = (base + S * p + i) // HU
            wmat[p, s * K + units] = 1
            s += 1
        base += 128 * S
    wmat[p, NSUB * K + p % (128 // RUNT)] = 1
    return wmat


_cache = {}


def _get_program():
    if "nc" not in _cache:
        _cache["nc"] = build_program()
        _cache["wmat"] = make_wmat()
    return _cache["nc"], _cache["wmat"]


def make_in_maps(masks):
    masks = np.ascontiguousarray(np.asarray(masks, dtype=np.float32))
    _, wmat = _get_program()
    rows = masks.reshape(N_CORES, R, W)
    return [{"masks": rows[c], "wmat": wmat} for c in range(N_CORES)]


def postprocess(results):
    """Per-core any-bitmaps -> boxes [N, 2, 2] f32 (exact integer math)."""
    nu = N_CORES * K  # 200 units (half masks)
    u_ymin = np.full(nu, float(H))
    u_ymax = np.full(nu, -1.0)
    u_xmin = np.full(nu, float(W))
    u_xmax = np.full(nu, -1.0)
    ys = np.arange(HU)
    xs = np.arange(W)
    for c, r in enumerate(results):
        ra = np.asarray(r["rowany_out"], np.float32)
        rows_any = np.empty(R, bool)
        s = 0
        base = 0
        for S in CHUNKS:
            rows_any[base : base + 128 * S] = (ra[:, s : s + S] > 0).reshape(-1)
            s += S
            base += 128 * S
        rows_any[base:] = (ra[:, NSUB] > 0).reshape(RUNT, 128 // RUNT).any(1)
        ca = np.asarray(r["colany_out"], np.float32) > 0  # [K, W]
        ca[K - 1] |= (np.asarray(r["runt_out"], np.float32) > 0).reshape(W)

        A = rows_any.reshape(K, HU)
        g = c * K + np.arange(K)
        off = (g % 2) * HU  # row offset of this unit within its mask
        has = A.any(1)
        u_ymin[g] = np.where(has, off + np.where(A, ys, H).min(1), H)
        u_ymax[g] = np.where(has, off + np.where(A, ys, -1).max(1), -1)
        hasx = ca.any(1)
        u_xmin[g] = np.where(hasx, np.where(ca, xs, W).min(1), W)
        u_xmax[g] = np.where(hasx, np.where(ca, xs, -1).max(1), -1)

    boxes = np.empty((N, 2, 2), np.float32)
    boxes[:, 0, 0] = u_xmin.reshape(N, 2).min(1)
    boxes[:, 0, 1] = u_ymin.reshape(N, 2).min(1)
    boxes[:, 1, 0] = u_xmax.reshape(N, 2).max(1)
    boxes[:, 1, 1] = u_ymax.reshape(N, 2).max(1)
    return boxes


def kernel(masks):
    nc, _ = _get_program()
    in_maps = make_in_maps(masks)
    res = run_bass_kernel_spmd(nc, in_maps, core_ids=list(range(N_CORES)))
    return postprocess(res.results)
